# revision 26
# baseline (speedup 1.0000x reference)
"""Trainium2 Bass kernel for CLIPAttention (B=32, S=512, E=768, H=12, D=64).

Strategy: data-parallel over batch across 8 NeuronCores (4 batches/core).
All matmul operands fp16 (PSUM accumulates fp32); hidden_states and weights
are pre-cast to fp16 on the host.

v2 changes over the 322 µs baseline (PE was 83% busy with ~60 µs at half
clock from HAM throttling during idle-ish attention phases):
  - x transpose moved off the PE onto the DMA xbar (dma_start_transpose
    straight from DRAM): kills 24 PE transposes + 8 PSUM->SBUF copies/batch.
  - softmax denominator folded into the PV matmul by augmenting each head's
    V block with a ones column (lhsT [128, 96]; output row 0 = den):
    kills 48 denominator matmuls/batch (~6.4 µs PE per batch).
  - causal mask of the 4 diagonal blocks applied in ONE strided DVE op per
    head instead of 4.
  - software-pipelined emission: attention heads of batch b are interleaved
    with out-proj of batch b-1 and the projections of batch b+1 so the PE
    never idles long enough for the HAM clock gate to drop to half rate.

Per batch:
  xt (feature-major, via DMA xbar) -> qT/kT feature-major + v token-major
  (v scattered so each head's 64 columns sit at stride 65 with a ones col).
  Attention per head with TRANSPOSED scores (k-major) so probabilities
  never need transposing:
    scoresT[k,q] = kh.T @ qh  (PE, triangular: only blocks k <= q)
    pE = exp(scale*scoresT)   (ACT, straight to SBUF fp16)
    diagonal blocks masked by one strided multiply with 0/1 upper-tri tiles
    po[32:96] = v_h.T @ pE ; po[0] = den   (PE, ones-augmented, triangular)
    rden = approx-recip(po[0])   (DVE) -> broadcast to 128 parts (GPSIMD)
    outT = po[32:96] * rden      (DVE, fused into the PSUM->SBUF copy)
  Final projection token-major; biases folded into PSUM->SBUF copies.
"""

import os
import time

import numpy as np
from contextlib import ExitStack

import concourse.bass as bass
import concourse.mybir as mybir
import concourse.tile as tile
from concourse import bacc
from concourse.bass_utils import run_bass_kernel_spmd
from concourse.masks import make_upper_triangular

B, S, E, H, D = 32, 512, 768, 12, 64
WB = 128                  # augmented v block: [ones | 63 pad | v(64)]
VOFF = 64                 # v data offset within the block (the BIR verifier
                          # requires 64-partition engine reads to start at
                          # partition 0 or 64)
NCORES = 8
NB = B // NCORES          # batches per core
P = 128
KT = E // P               # 6 feature tiles
QT = S // P               # 4 token tiles
SCALE = float(D) ** -0.5  # 0.125
F32 = mybir.dt.float32
F16 = mybir.dt.float16

AF = mybir.ActivationFunctionType
OP = mybir.AluOpType

NSPLIT = 384              # N-tile for the token-major projections
HN = NSPLIT // D          # heads per N-chunk = 6


def _build():
    nc = bacc.Bacc(trn_type="TRN2")

    hs = nc.dram_tensor("hs", [NB, S, E], F16, kind="ExternalInput")
    w_dr = {}
    b_dr = {}
    for nm in ("q", "k", "v", "o"):
        # host-staged block layout: [partition, out-col block m, contraction
        # block ko, 128] so each partition's bytes per DMA are contiguous and
        # the weight can stream in m-block chunks ahead of its consumers
        w_dr[nm] = nc.dram_tensor(f"W{nm}", [P, KT, KT, P], F16, kind="ExternalInput")
        b_dr[nm] = nc.dram_tensor(f"b{nm}", [E], F32, kind="ExternalInput")
    out = nc.dram_tensor("out", [NB, S, E], F32, kind="ExternalOutput")

    with ExitStack() as ctx:
        tc = ctx.enter_context(tile.TileContext(nc))

        singles = ctx.enter_context(tc.tile_pool(name="singles", bufs=1))
        xtpool = ctx.enter_context(tc.tile_pool(name="xtpool", bufs=2))
        qkvpool = ctx.enter_context(tc.tile_pool(name="qkvpool", bufs=2))
        pepool = ctx.enter_context(tc.tile_pool(name="pepool", bufs=3))
        rpool = ctx.enter_context(tc.tile_pool(name="rpool", bufs=2))
        otpool = ctx.enter_context(tc.tile_pool(name="otpool", bufs=2))
        opool = ctx.enter_context(tc.tile_pool(name="opool", bufs=2))

        ps_s = ctx.enter_context(tc.tile_pool(name="ps_s", bufs=4, space="PSUM"))
        ps_mm = ctx.enter_context(tc.tile_pool(name="ps_mm", bufs=2, space="PSUM"))
        ps_pv = ctx.enter_context(tc.tile_pool(name="ps_pv", bufs=2, space="PSUM"))

        # ---- constants ----
        # 0/1 upper-triangular (incl diagonal) masks, one per diagonal block,
        # so all 4 blocks of a head are masked in a single strided DVE op
        triu4 = singles.tile([P, QT, P], F16, name="triu4")
        for j in range(QT):
            make_upper_triangular(nc, triu4[:, j, :], val=1.0, diag=True)

        # ---- input / weight DMAs (x for batch 0 first so compute starts) ----
        xt_t = {}

        def emit_xt_dma(b, eng=None):
            # token-chunked layout [P, QT, KT, P]: each chunk's transpose has
            # a contiguous SBUF destination, so batch 0's first projections
            # can start on half an xt
            xt = xtpool.tile([P, QT, KT, P], F16, name=f"xt_{b}", tag="xt")
            e = eng or nc.sync
            for i in range(QT):
                e.dma_start_transpose(out=xt[:, i], in_=hs[b, i * P:(i + 1) * P, :])
            xt_t[b] = xt

        # xt0 alone on the scalar HWDGE queue; weights stream on the SP
        # queue in 2-m-block chunks in consumption order, so the first
        # matmul waits only for xt0's first half and Wq's first chunk.
        # The gpsimd queue stays empty: DMA triggers there would
        # head-of-line block the partition broadcasts.
        emit_xt_dma(0, nc.scalar)

        bias_pp = {}
        bias_bc = {}
        w_sb = {}
        for nm in ("q", "k", "v", "o"):
            w_sb[nm] = singles.tile([P, KT, KT, P], F16, name=f"w_{nm}")

        def emit_w_blocks(nm):
            for t in range(KT // 2):
                nc.sync.dma_start(
                    out=w_sb[nm][:, 2 * t:2 * t + 2],
                    in_=w_dr[nm][:, 2 * t:2 * t + 2],
                )

        def emit_bias_pp(nm):
            # per-partition bias form for feature-major q/k (host-permuted)
            bias_pp[nm] = singles.tile([P, KT], F32, name=f"bpp_{nm}")
            nc.sync.dma_start(
                out=bias_pp[nm], in_=b_dr[nm].rearrange("(p ko) -> p ko", ko=KT)
            )

        def emit_bias_bc(nm):
            bias_bc[nm] = singles.tile([P, E], F32, name=f"bbc_{nm}")
            src = b_dr[nm][:]
            bcast = bass.AP(
                tensor=src.tensor, offset=src.offset, ap=[[0, P], *src.ap]
            )
            nc.sync.dma_start(out=bias_bc[nm], in_=bcast)

        emit_w_blocks("q")
        emit_bias_pp("q")
        emit_bias_pp("k")
        emit_w_blocks("k")
        emit_xt_dma(1)
        emit_bias_bc("v")
        emit_w_blocks("v")
        emit_w_blocks("o")
        emit_bias_bc("o")

        # ---- per-batch stage emitters -----------------------------------
        qkv_t = {}     # b -> {"q": tile, "k": tile}
        vplus_t = {}   # b -> ones-augmented v tile [P, QT, H*D1]
        outT_t = {}    # b -> attention output, feature-major
        pE_live = {}

        def b_unit(b, nm, m):
            # one m-tile of the q/k projection, feature-major output. The
            # first units of batch 0 run in token-halves so they only wait
            # for half of xt0 at startup.
            if m == 0:
                dst = qkvpool.tile([P, KT, S], F16, name=f"{nm}T_{b}", tag=f"{nm}T")
                qkv_t.setdefault(b, {})[nm] = dst
            dst = qkv_t[b][nm]
            ps = ps_mm.tile([P, S], F32, name=f"ps{nm}_{b}_{m}", tag="mm")
            halves = 2 if (b == 0 and nm == "q" and m < 2) else 1
            for hf in range(halves):
                nh = QT // halves
                for kk in range(KT):
                    nc.tensor.matmul(
                        ps[:, hf * nh * P:(hf + 1) * nh * P],
                        lhsT=w_sb[nm][:, m, kk, :],
                        rhs=xt_t[b][:, hf * nh:(hf + 1) * nh, kk, :],
                        start=(kk == 0),
                        stop=(kk == KT - 1),
                    )
            if m % 2 == 0:
                nc.scalar.activation(
                    out=dst[:, m, :],
                    in_=ps,
                    func=AF.Identity,
                    bias=bias_pp[nm][:, m:m + 1],
                    scale=1.0,
                )
            else:
                nc.vector.tensor_scalar_add(
                    out=dst[:, m, :], in0=ps, scalar1=bias_pp[nm][:, m:m + 1]
                )

        def c_unit(b, i, n):
            # one (token-tile, n-chunk) of the v projection, scattered so head
            # h' of the chunk lands at columns h'*65..h'*65+63 (ones at +64)
            if i == 0 and n == 0:
                vp = qkvpool.tile([P, QT, H * WB], F16, name=f"v_{b}", tag="v")
                vplus_t[b] = vp
                if b < 2:
                    # fill the whole (contiguous) tile with 1.0 once per pool
                    # buffer; the data writes below leave only the ones
                    # columns at 1.0, giving the denominator row of the
                    # ones-augmented PV matmul
                    nc.vector.memset(vp[:], 1.0)
            vp = vplus_t[b]
            ps = ps_mm.tile([P, S], F32, name=f"psv_{b}_{i}_{n}", tag="mm")
            for kk in range(KT):
                nc.tensor.matmul(
                    ps[:, :NSPLIT],
                    lhsT=xt_t[b][:, i, kk, :],
                    rhs=w_sb["v"][:, 3 * n:3 * n + 3, kk, :],
                    start=(kk == 0),
                    stop=(kk == KT - 1),
                )
            # ones column FIRST in each head block: the denominator then lands
            # at partition 0 of the PV output, where reciprocal_approx_fast
            # reads correctly (its custom-DVE lowering mishandles nonzero
            # input base partitions); v data at +32 keeps the normalize read
            # partition-aligned
            dst = vp[:, i, n * HN * WB:(n + 1) * HN * WB].rearrange(
                "p (h d) -> p h d", d=WB
            )[:, :, VOFF:VOFF + D]
            nc.vector.tensor_tensor(
                out=dst,
                in0=ps[:, :NSPLIT].rearrange("p (h d) -> p h d", d=D),
                in1=bias_bc["v"][:, n * NSPLIT:(n + 1) * NSPLIT].rearrange(
                    "p (h d) -> p h d", d=D
                ),
                op=OP.add,
            )

        def d_scores(b, h):
            # k-major scores + exp for one head; diagonal blocks masked in
            # one strided DVE multiply
            g, rr = h // 2, h % 2
            qh = qkv_t[b]["q"][rr * D:(rr + 1) * D, g, :]
            kh = qkv_t[b]["k"][rr * D:(rr + 1) * D, g, :]
            pE = pepool.tile([P, QT, S], F16, name=f"pE_{b}_{h}", tag="pE")
            pE_live[(b, h)] = pE
            for j in range(QT):
                q0 = j * P
                ps = ps_s.tile([P, S], F32, name=f"pss_{b}_{h}_{j}", tag="s")
                nc.tensor.matmul(
                    ps[:, :S - q0],
                    lhsT=kh[:, q0:q0 + P],
                    rhs=qh[:, q0:],
                    start=True,
                    stop=True,
                )
                nc.scalar.activation(
                    out=pE[:, j, q0:],
                    in_=ps[:, :S - q0],
                    func=AF.Exp,
                    scale=SCALE,
                )
            base = pE[:, 0, 0:P]
            diag = bass.AP(
                tensor=base.tensor,
                offset=base.offset,
                ap=[base.ap[0], [S + P, QT], [1, P]],
            )
            nc.vector.tensor_tensor(out=diag, in0=diag, in1=triu4[:], op=OP.mult)

        def d_pv(b, h):
            # ones-augmented PV: row 64 of po is the softmax denominator
            g, rr = h // 2, h % 2
            pE = pE_live.pop((b, h))
            po = ps_pv.tile([WB, S], F32, name=f"po_{b}_{h}", tag="pv")
            for j in range(QT):
                nc.tensor.matmul(
                    po[:, j * P:],
                    lhsT=vplus_t[b][:, j, h * WB:(h + 1) * WB],
                    rhs=pE[:, j, j * P:],
                    start=(j == 0),
                    stop=(j == QT - 1),
                    skip_group_check=True,
                )
            rden = rpool.tile([1, S], F32, name=f"rden_{b}_{h}", tag="rden")
            nc.vector.reciprocal_approx_fast(rden, po[0:1, :])
            rb = rpool.tile([P, S], F32, name=f"rb_{b}_{h}", tag="rb")
            nc.gpsimd.partition_broadcast(rb, rden)

            def norm(b=b, h=h, g=g, rr=rr, po=po, rb=rb):
                nc.vector.tensor_tensor(
                    out=outT_t[b][rr * D:(rr + 1) * D, g, :],
                    in0=po[VOFF:VOFF + D, :],
                    in1=rb[0:D, :],
                    op=OP.mult,
                )

            # defer the normalize one head so the DVE is never head-of-line
            # blocked waiting on the GPSIMD broadcast
            if d_pv.norm_pending is not None:
                d_pv.norm_pending()
            d_pv.norm_pending = norm

        d_pv.norm_pending = None

        def e_unit(b, i, n):
            # one (token-tile, n-chunk) of the out projection (+ DMA on n==1)
            key = (b, i)
            if n == 0:
                o_t = opool.tile([P, E], F32, name=f"o_{b}_{i}", tag="o")
                e_unit.o_t[key] = o_t
            o_t = e_unit.o_t[key]
            ps = ps_mm.tile([P, S], F32, name=f"pso_{b}_{i}_{n}", tag="mm")
            for kk in range(KT):
                nc.tensor.matmul(
                    ps[:, :NSPLIT],
                    lhsT=outT_t[b][:, kk, i * P:(i + 1) * P],
                    rhs=w_sb["o"][:, 3 * n:3 * n + 3, kk, :],
                    start=(kk == 0),
                    stop=(kk == KT - 1),
                )
            nc.vector.tensor_tensor(
                out=o_t[:, n * NSPLIT:(n + 1) * NSPLIT],
                in0=ps[:, :NSPLIT],
                in1=bias_bc["o"][:, n * NSPLIT:(n + 1) * NSPLIT],
                op=OP.add,
            )
            if n == 1:
                # last batch: alternate queues so the final drain parallelizes
                # (the scalar queue is idle by then)
                eng = nc.scalar if (b == NB - 1 and i % 2 == 0) else nc.sync
                eng.dma_start(out=out[b, i * P:(i + 1) * P, :], in_=o_t)
                del e_unit.o_t[key]

        e_unit.o_t = {}

        def proj_units(b):
            for nm in ("q", "k"):
                for m in range(KT):
                    yield lambda b=b, nm=nm, m=m: b_unit(b, nm, m)
            for i in range(QT):
                for n in range(E // NSPLIT):
                    yield lambda b=b, i=i, n=n: c_unit(b, i, n)

        # ---- software-pipelined emission --------------------------------
        # startup: batch 0 projections un-overlapped (pipeline fill)
        for u in proj_units(0):
            u()

        for b in range(NB):
            outT_t[b] = otpool.tile([P, KT, S], F16, name=f"outT_{b}", tag="outT")
            fillers = []
            if b >= 1:
                fillers += [
                    (lambda b2=b - 1, i=i, n=n: e_unit(b2, i, n))
                    for i in range(QT)
                    for n in range(E // NSPLIT)
                ]
            if b + 1 < NB:
                if b + 2 < NB:
                    fillers.append(lambda b2=b + 2: emit_xt_dma(b2))
                fillers += list(proj_units(b + 1))

            nf = len(fillers)
            done = 0
            d_scores(b, 0)
            for h in range(H):
                if h + 1 < H:
                    d_scores(b, h + 1)
                d_pv(b, h)
                # spread fillers across the head rounds
                want = (nf * (h + 1)) // H
                while done < want:
                    fillers[done]()
                    done += 1
            if d_pv.norm_pending is not None:
                d_pv.norm_pending()
                d_pv.norm_pending = None

        for i in range(QT):
            for n in range(E // NSPLIT):
                e_unit(NB - 1, i, n)

    nc.compile()
    return nc


_NC_CACHE = None


def _get_nc():
    global _NC_CACHE
    if _NC_CACHE is None:
        _NC_CACHE = _build()
    return _NC_CACHE


def run(inputs, trace=False):
    if trace:
        os.environ.pop("BASS_NEVER_TRACE", None)
    else:
        # keep the spmd runner off the NTFF trace path (the profiling hook
        # module is not always present)
        os.environ["BASS_NEVER_TRACE"] = "1"
    # hidden_states and weights are pre-cast to fp16 on the host: identical
    # rounding to the on-chip cast, but half the DMA bytes and no staging
    hs = np.ascontiguousarray(
        np.asarray(inputs["hidden_states"], dtype=np.float32).astype(np.float16)
    )
    assert hs.shape == (B, S, E)
    # permute contraction rows so row p*KT+ko of the staged weight is row
    # ko*128+p of the original: the device DMA then reads one contiguous
    # 9216B run per partition (descriptor-bound otherwise)
    wb = {}
    for nm in ("q", "k", "v", "o"):
        w = np.asarray(inputs[f"W{nm}"], dtype=np.float32).astype(np.float16)
        # [P, m-block, ko-block, 128]: W_stage[p, m, ko, d] = W[ko*128+p, m*128+d]
        wb[f"W{nm}"] = np.ascontiguousarray(
            w.reshape(KT, P, KT, P).transpose(1, 2, 0, 3)
        )
        bv = np.asarray(inputs[f"b{nm}"], dtype=np.float32)
        if nm in ("q", "k"):
            bv = np.ascontiguousarray(bv.reshape(KT, P).T.reshape(E))
        wb[f"b{nm}"] = np.ascontiguousarray(bv)

    nc = _get_nc()
    in_maps = []
    for c in range(NCORES):
        m = {"hs": hs[c * NB:(c + 1) * NB]}
        m.update(wb)
        in_maps.append(m)
    res = run_bass_kernel_spmd(
        nc, in_maps, core_ids=list(range(NCORES)), trace=trace
    )
    outp = np.concatenate([r_["out"] for r_ in res.results], axis=0)
    return outp, res


def kernel(**inputs) -> np.ndarray:
    # retry once on transient accelerator errors (rare NRT exec glitches)
    last = None
    for attempt in range(2):
        try:
            outp, _ = run(inputs, trace=False)
            return outp
        except Exception as e:  # noqa: BLE001
            last = e
            time.sleep(10)
    raise last
